# revision 23
# baseline (speedup 1.0000x reference)
"""BlockTransformerPairBias Trainium2 kernel (v2 — pipelined).

Sharding: 8 cores = (batch 0/1) x (4 groups of 16 attention blocks).
Each core computes its 1024 tokens end-to-end; no collectives.

v2 restructure vs baseline:
- PSUM/SBUF double buffering everywhere (no bufs=1 serialization)
- batched LN statistics; ACT pinned to one transcendental per phase
  (table reloads cost 46us in the baseline); PSUM->SBUF copies ride
  ACT's Copy path (no table) or DVE
- bias-path weights pre-folded (wbs' = w - S/128) so the pair bias is
  just P'*rstd; the P reshape round-trip is batched per 8-block chunk
- attention software-pipelined across block-pairs (QK of gp+1 emitted
  before the tail of gp); V projection 2-way column-tiled
- re/h kept in bf16; zT loads split across sync+gpsimd DMA queues
"""

import sys

sys.path.insert(0, "/opt/trn_rl_repo")

from contextlib import ExitStack

import numpy as np
import ml_dtypes

import concourse.bass as bass
import concourse.tile as tile
from concourse import bacc, mybir
from concourse.bass_utils import run_bass_kernel_spmd
from concourse.masks import make_identity
from concourse.tile import add_dep_helper

F32 = mybir.dt.float32
BF16 = mybir.dt.bfloat16
I16 = mybir.dt.int16
AF = mybir.ActivationFunctionType
ALU = mybir.AluOpType
BF = ml_dtypes.bfloat16

B, N, NRES = 2, 4096, 1024
CS, CC, CZ, H, BLK = 512, 384, 128, 8, 64
CH = CS // H          # 64
NB = N // BLK         # 64
NCORES = 8
NBLK = NB * B // NCORES   # 16 blocks per core
NT = NBLK * BLK           # 1024 tokens per core
RT = NT // 128            # 8 token tiles
EPS = 1e-5

_CACHE = {}


def _declare(nc):
    t = {}

    def inp(name, shape, dt):
        t[name] = nc.dram_tensor(name, list(shape), dt, kind="ExternalInput").ap()

    inp("re", (NT, CS), BF16)
    inp("zT", (NBLK, CZ, BLK * BLK), BF16)
    inp("s", (NRES, CC), F32)
    inp("idx", (128, NT // 16), I16)
    inp("wq", (128, 4, CS), BF16)
    inp("wk", (128, 4, CS), BF16)
    inp("wv", (128, 4, CS), BF16)
    inp("wg", (128, 4, CS), BF16)
    inp("wout", (128, 4, CS), BF16)
    inp("w1", (128, 4, 2 * CS), BF16)
    inp("w2", (128, 4, 2 * CS), BF16)
    inp("wb", (128, 8, CS), BF16)
    inp("wada", (128, 3, 3 * CS), BF16)
    inp("wbs", (CZ, 64), BF16)
    inp("bq", (128, 4), F32)
    inp("bk", (128, 4), F32)
    inp("bada", (3 * CS,), F32)
    t["out"] = nc.dram_tensor("out", [NT, CS], F32, kind="ExternalOutput").ap()
    return t


def _bcast(ap, p=128):
    """Broadcast a 1-D DRAM AP across p partitions."""
    return bass.AP(tensor=ap.tensor, offset=ap.offset, ap=[[0, p]] + list(ap.ap))


def _b0(ap_, reps, at=None):
    """Insert a 0-stride broadcast dim into a free position of an AP."""
    lst = list(ap_.ap)
    pos = len(lst) if at is None else at
    lst.insert(pos, [0, reps])
    return bass.AP(tensor=ap_.tensor, offset=ap_.offset, ap=lst)


def _batch_stats(nc, sb, src_rc, nrt, tagp):
    """LN stats for nrt row-tiles: returns (mv [128,nrt,2], rstd [128,nrt])."""
    st6 = sb.tile([128, nrt, 6], F32, tag=tagp + "st6", name="st6")
    for r in range(nrt):
        nc.vector.bn_stats(st6[:, r, :], src_rc(r))
    mv = sb.tile([128, nrt, 2], F32, tag=tagp + "mv", name="mv")
    for r in range(nrt):
        nc.vector.bn_aggr(mv[:, r, :], st6[:, r, :])
    vc = sb.tile([128, nrt], F32, tag=tagp + "vc", name="vc")
    nc.vector.tensor_scalar_add(
        vc[:], mv[:, :, 1:2].rearrange("p r o -> p (r o)"), EPS)
    rv = sb.tile([128, nrt], F32, tag=tagp + "rv", name="rv")
    nc.vector.reciprocal_approx_fast(out=rv[:], in_=vc[:])
    rstd = sb.tile([128, nrt], F32, tag=tagp + "rstd", name="rstd")
    nc.scalar.activation(rstd[:], rv[:], AF.Sqrt)
    return mv, rstd


def _emit(ctx, tc, t, flags):
    nc = tc.nc
    has_bq, has_bk, has_bag, btg_const, has_btg = flags

    consts = ctx.enter_context(tc.tile_pool(name="consts", bufs=1))
    top = ctx.enter_context(tc.tile_pool(name="top", bufs=1))
    sb = ctx.enter_context(tc.tile_pool(name="sb", bufs=2))
    dramp = ctx.enter_context(tc.tile_pool(name="dram", bufs=1, space="DRAM"))

    # ---- early DMAs: re first (B stats start immediately), then weights ----
    ident = consts.tile([128, 128], BF16)
    make_identity(nc, ident[:])
    re_sb = top.tile([128, RT, CS], BF16)
    re_src = t["re"].rearrange("(r p) c -> p r c", p=128)
    for r in range(RT):
        nc.sync.dma_start(re_sb[:, r, :], re_src[:, r, :])
    idx_sb = consts.tile([128, NT // 16], I16)
    nc.sync.dma_start(idx_sb[:], t["idx"][:])
    wbs_sb = consts.tile([CZ, 64], BF16)
    nc.sync.dma_start(wbs_sb[:], t["wbs"][:])
    bq_sb = consts.tile([128, 4], F32)
    bk_sb = consts.tile([128, 4], F32)
    if has_bq:
        nc.sync.dma_start(bq_sb[:], t["bq"][:])
    if has_bk:
        nc.sync.dma_start(bk_sb[:], t["bk"][:])
    wq = consts.tile([128, 4, CS], BF16)
    nc.sync.dma_start(wq[:], t["wq"][:])
    wk = consts.tile([128, 4, CS], BF16)
    nc.sync.dma_start(wk[:], t["wk"][:])
    wv = consts.tile([128, 4, CS], BF16)
    nc.sync.dma_start(wv[:], t["wv"][:])
    wg = consts.tile([128, 4, CS], BF16)
    nc.sync.dma_start(wg[:], t["wg"][:])
    wout = consts.tile([128, 4, CS], BF16)
    nc.sync.dma_start(wout[:], t["wout"][:])
    btg_t = consts.tile([128, 1], F32)
    if btg_const:
        nc.vector.memset(btg_t[:], btg_const)

    h_sb = top.tile([128, RT, CS], BF16)
    gth = top.tile([128, RT, 3 * CS], BF16)

    tbl = dramp.tile([NRES, 3 * CS], BF16)
    pr_d = dramp.tile([10, NBLK * BLK * BLK], BF16)

    bd_stack = ExitStack()
    mid = bd_stack.enter_context(tc.tile_pool(name="mid", bufs=1))
    qf = mid.tile([128, 4, NT], BF16)
    kf = mid.tile([128, 4, NT], BF16)
    qf2 = mid.tile([64, 4, NT], BF16)
    kf2 = mid.tile([64, 4, NT], BF16)
    vtm = mid.tile([128, RT, CS], BF16)
    gsig = mid.tile([128, RT, CS], BF16)
    cdp = bd_stack.enter_context(tc.tile_pool(name="cdp", bufs=2))
    bc_stack = ExitStack()
    xnp = bc_stack.enter_context(tc.tile_pool(name="xnp", bufs=1))
    xnT = xnp.tile([128, 4, NT], BF16)

    # ---- B stats + xnT first: PE gets work within ~6us ----
    with tc.tile_pool(name="bx", bufs=2) as bx, \
         tc.tile_pool(name="ps_tp", bufs=2, space="PSUM") as ps_tp:
        mvb, rstdb = _batch_stats(nc, sb, lambda r: re_sb[:, r, :], RT, "bb")
        for r in range(RT):
            xn = bx.tile([128, CS], BF16, tag="xn")
            nc.vector.tensor_scalar(out=xn[:], in0=re_sb[:, r, :],
                                    scalar1=mvb[:, r, 0:1],
                                    scalar2=rstdb[:, r:r + 1],
                                    op0=ALU.subtract, op1=ALU.mult)
            tp = ps_tp.tile([128, CS], BF16, tag="tp")
            for c in range(4):
                nc.tensor.transpose(tp[:, c * 128:(c + 1) * 128],
                                    xn[:, c * 128:(c + 1) * 128], ident[:])
            nc.vector.tensor_copy(
                xnT[:, :, r * 128:(r + 1) * 128],
                tp[:].rearrange("p (k c) -> p k c", k=4))

    # ================= P1: cond tables =================
    with tc.tile_pool(name="p1", bufs=2) as p1p, \
         tc.tile_pool(name="p1w", bufs=1) as p1w, \
         tc.tile_pool(name="ps_tp1", bufs=2, space="PSUM") as ps_tp1, \
         tc.tile_pool(name="ps_p1", bufs=3, space="PSUM") as ps_p1:
        wada = p1w.tile([128, 3, 3 * CS], BF16)
        nc.sync.dma_start(wada[:], t["wada"][:])
        bada_bc = p1w.tile([128, 3 * CS], F32)
        if has_bag or (has_btg and btg_const is None):
            nc.sync.dma_start(bada_bc[:], _bcast(t["bada"]))
        s_all = p1w.tile([128, NRES // 128, CC], F32)
        nc.gpsimd.dma_start(s_all[:], t["s"].rearrange("(r p) c -> p r c", p=128))

        mv1, rstd1 = _batch_stats(nc, sb, lambda r: s_all[:, r, :], 8, "p1")
        for r in range(NRES // 128):
            cond = p1p.tile([128, CC], BF16, tag="cond")
            nc.vector.tensor_scalar(out=cond[:], in0=s_all[:, r, :],
                                    scalar1=mv1[:, r, 0:1],
                                    scalar2=rstd1[:, r:r + 1],
                                    op0=ALU.subtract, op1=ALU.mult)
            tp = ps_tp1.tile([128, CC], BF16, tag="tp1")
            for c in range(3):
                nc.tensor.transpose(tp[:, c * 128:(c + 1) * 128],
                                    cond[:, c * 128:(c + 1) * 128], ident[:])
            ct = p1p.tile([128, 3, 128], BF16, tag="ct")
            nc.vector.tensor_copy(ct[:].rearrange("p k c -> p (k c)"), tp[:])
            tbl_sb = p1p.tile([128, 3 * CS], BF16, tag="tblsb")
            for n in range(3):
                pt = ps_p1.tile([128, CS], F32, tag="p1pt")
                for k in range(3):
                    nc.tensor.matmul(pt[:], ct[:, k, :],
                                     wada[:, k, n * CS:(n + 1) * CS],
                                     start=(k == 0), stop=(k == 2))
                seg = slice(n * CS, (n + 1) * CS)
                if n == 0:
                    if has_bag:
                        nc.vector.tensor_add(pt[:], pt[:], bada_bc[:, seg])
                    nc.scalar.activation(tbl_sb[:, seg], pt[:], AF.Sigmoid)
                elif n == 1:
                    nc.vector.tensor_copy(tbl_sb[:, seg], pt[:])
                else:
                    if has_btg and btg_const is None:
                        nc.vector.tensor_add(pt[:], pt[:], bada_bc[:, seg])
                        nc.scalar.activation(tbl_sb[:, seg], pt[:], AF.Sigmoid)
                    elif btg_const:
                        nc.scalar.activation(tbl_sb[:, seg], pt[:], AF.Sigmoid,
                                             bias=btg_t[:])
                    else:
                        nc.scalar.activation(tbl_sb[:, seg], pt[:], AF.Sigmoid)
            nc.gpsimd.dma_start(tbl[r * 128:(r + 1) * 128, :], tbl_sb[:])

    # gathers right after P1 on the gpsimd queue (consumed in E)
    for r in range(RT):
        nc.gpsimd.dma_gather(
            out_ap=gth[:, r:r + 1, :], in_ap=tbl[:],
            idxs_ap=idx_sb[:, r * 8:(r + 1) * 8],
            num_idxs=128, num_idxs_reg=128, elem_size=3 * CS)

    # ====== B projections interleaved with C bias-path blocks ======
    pallp = bc_stack.enter_context(tc.tile_pool(name="pallp", bufs=2))
    ztp = bc_stack.enter_context(tc.tile_pool(name="ztp", bufs=2))
    z2p = bc_stack.enter_context(tc.tile_pool(name="z2p", bufs=1))

    ps_pt = bc_stack.enter_context(tc.tile_pool(name="ps_pt", bufs=3, space="PSUM"))
    ps_ze = bc_stack.enter_context(tc.tile_pool(name="ps_ze", bufs=2, space="PSUM"))
    ps_zo = bc_stack.enter_context(tc.tile_pool(name="ps_zo", bufs=2, space="PSUM"))

    Pall = {0: None, 1: None}

    def emit_c_block(g):
        ch, gl = g // 8, g % 8
        if gl == 0:
            Pall[ch] = pallp.tile([128, 8, 1024], BF16, tag="Pall", name="Pall")
        zt_t = ztp.tile([128, BLK * BLK], BF16, tag="zt")
        nc.sync.dma_start(zt_t[:], t["zT"][g])
        z2 = z2p.tile([128, BLK * BLK], BF16, tag="z2")
        nc.vector.tensor_mul(z2[:], zt_t[:], zt_t[:])
        ze = ps_ze.tile([128, 512], F32, tag="ze")
        zo = ps_zo.tile([128, 512], F32, tag="zo")
        for cg in range(4):
            tpos = (0, 32 * cg)
            rows = slice(32 * cg, 32 * cg + 32)
            ev = slice((2 * cg) * 512, (2 * cg + 1) * 512)
            od = slice((2 * cg + 1) * 512, (2 * cg + 2) * 512)
            nc.tensor.matmul(ze[rows, :], wbs_sb[:, 0:32], zt_t[:, ev],
                             start=True, stop=False, tile_position=tpos)
            nc.tensor.matmul(zo[rows, :], wbs_sb[:, 0:32], zt_t[:, od],
                             start=True, stop=False, tile_position=tpos)
            nc.tensor.matmul(ze[rows, :], wbs_sb[:, 32:64], z2[:, ev],
                             start=False, stop=True, tile_position=tpos)
            nc.tensor.matmul(zo[rows, :], wbs_sb[:, 32:64], z2[:, od],
                             start=False, stop=True, tile_position=tpos)
        nc.scalar.copy(Pall[ch][:, gl, 0:512], ze[:])
        nc.scalar.copy(Pall[ch][:, gl, 512:1024], zo[:])

    def emit_c_roundtrip(ch):
        # reshape round-trip P[32cg+m, (strip w)] -> pr_d[m, g, i, j];
        # DMAs ride the scalar queue right behind the Pall copies
        prt = pr_d[:]
        sts = []
        for cg in range(4):
            for p2 in range(2):
                src = Pall[ch][32 * cg:32 * cg + 10, :, p2 * 512:(p2 + 1) * 512]
                dst = bass.AP(
                    tensor=prt.tensor,
                    offset=prt.offset + ch * 32768 + (2 * cg + p2) * 512,
                    ap=[[65536, 10], [4096, 8], [1, 512]])
                sts.append(nc.scalar.dma_start(dst, src))
        Pr_sb = cdp.tile([128, 4, 640], BF16, tag="Pr")
        for gl in range(4):
            src = bass.AP(tensor=prt.tensor,
                          offset=prt.offset + ch * 32768 + gl * 8192,
                          ap=[[64, 128], [65536, 10], [1, 64]])
            ld = nc.scalar.dma_start(
                Pr_sb[:, gl, :].rearrange("p (m j) -> p m j", m=10), src)
            for st in sts:
                add_dep_helper(ld.ins, st.ins, reason="pr RAW")
        msq = sb.tile([128, 4, 64], F32, tag="msq")
        nc.vector.tensor_mul(msq[:], Pr_sb[:, :, 512:576], Pr_sb[:, :, 512:576])
        var = sb.tile([128, 4, 64], F32, tag="var")
        nc.vector.scalar_tensor_tensor(out=var[:], in0=Pr_sb[:, :, 576:640],
                                       scalar=EPS, in1=msq[:],
                                       op0=ALU.add, op1=ALU.subtract)
        rv = sb.tile([128, 4, 64], F32, tag="rvc")
        nc.vector.reciprocal_approx_fast(out=rv[:], in_=var[:])
        rstd_c = cdp.tile([128, 4, 64], F32, tag="rstdc")
        nc.scalar.activation(rstd_c[:], rv[:], AF.Sqrt)
        return Pr_sb, rstd_c

    # Q
    for m in range(4):
        for n in range(2):
            pt = ps_pt.tile([128, CS], F32, tag="pt")
            for k in range(4):
                nc.tensor.matmul(pt[:], wq[:, k, m * 128:(m + 1) * 128],
                                 xnT[:, k, n * 512:(n + 1) * 512],
                                 start=(k == 0), stop=(k == 3))
            dseg = qf[:, m, n * 512:(n + 1) * 512]
            if has_bq:
                nc.vector.tensor_scalar_add(out=dseg, in0=pt[:],
                                            scalar1=bq_sb[:, m:m + 1])
            else:
                nc.vector.tensor_copy(dseg, pt[:])
    for g in range(0, 3):
        emit_c_block(g)
    # K
    for m in range(4):
        for n in range(2):
            pt = ps_pt.tile([128, CS], F32, tag="pt")
            for k in range(4):
                nc.tensor.matmul(pt[:], wk[:, k, m * 128:(m + 1) * 128],
                                 xnT[:, k, n * 512:(n + 1) * 512],
                                 start=(k == 0), stop=(k == 3))
            dseg = kf[:, m, n * 512:(n + 1) * 512]
            if has_bk:
                nc.vector.tensor_scalar_add(out=dseg, in0=pt[:],
                                            scalar1=bk_sb[:, m:m + 1])
            else:
                nc.vector.tensor_copy(dseg, pt[:])
    nc.sync.dma_start(qf2[:], qf[64:128, :, :])
    nc.sync.dma_start(kf2[:], kf[64:128, :, :])
    for g in range(3, 8):
        emit_c_block(g)
    PrA, rstdA = emit_c_roundtrip(0)
    # V (2-way column-tiled: block pair per PSUM tile)
    for gpair in range(RT):
        ptv = ps_pt.tile([128, CS], F32, tag="pt")
        for gg in range(2):
            g = 2 * gpair + gg
            for k in range(4):
                nc.tensor.matmul(ptv[gg * 64:gg * 64 + 64, :],
                                 xnT[:, k, g * 64:(g + 1) * 64],
                                 wv[:, k, :], start=(k == 0), stop=(k == 3),
                                 tile_position=(0, gg * 64))
        nc.scalar.copy(vtm[:, gpair, :], ptv[:])
    for g in range(8, 12):
        emit_c_block(g)
    # G
    for r in range(RT):
        ptg = ps_pt.tile([128, CS], F32, tag="pt")
        for k in range(4):
            nc.tensor.matmul(ptg[:], xnT[:, k, r * 128:(r + 1) * 128],
                             wg[:, k, :], start=(k == 0), stop=(k == 3))
        nc.scalar.activation(gsig[:, r, :], ptg[:], AF.Sigmoid)
    for g in range(12, 16):
        emit_c_block(g)
    PrB, rstdB = emit_c_roundtrip(1)
    bc_stack.close()

    # ================= D1: QK + softmax + transposed A =================
    dmid = bd_stack.enter_context(tc.tile_pool(name="dmid", bufs=1))
    aTs_all = dmid.tile([128, RT, CS], BF16)
    gr_all = dmid.tile([128, RT, CS], BF16)
    pr_of = lambda gp: (PrA, rstdA) if gp < 4 else (PrB, rstdB)

    def emit_d1_qk(gp, ps_sc):
        Pr_sb, rstd_c = pr_of(gp)
        gl = gp % 4
        pb = sb.tile([128, 8, 64], BF16, tag="pb")
        nc.vector.tensor_mul(pb[:],
                             Pr_sb[:, gl, 0:512].rearrange("p (h j) -> p h j", h=H),
                             _b0(rstd_c[:, gl, :], H, at=1))
        scp = ps_sc.tile([128, CS], F32, tag="sc")
        for h in range(H):
            m = h // 2
            for g2 in range(2):
                g = 2 * gp + g2
                qsl = (qf[0:64, m, g * 64:(g + 1) * 64] if h % 2 == 0
                       else qf2[:, m, g * 64:(g + 1) * 64])
                ksl = (kf[0:64, m, g * 64:(g + 1) * 64] if h % 2 == 0
                       else kf2[:, m, g * 64:(g + 1) * 64])
                nc.tensor.matmul(scp[g2 * 64:g2 * 64 + 64, h * 64:(h + 1) * 64],
                                 qsl, ksl, start=True, stop=True,
                                 tile_position=(0, g2 * 64))
        return pb, scp

    def emit_d1_rest(gp, pb, scp, ps_at_a, ps_at_b):
        a_sb = sb.tile([128, CS], BF16, tag="a_sb")
        nc.vector.tensor_add(a_sb[:].rearrange("p (h j) -> p h j", h=H),
                             scp[:].rearrange("p (h j) -> p h j", h=H), pb[:])
        ax = sb.tile([128, CS], BF16, tag="ax")
        nc.scalar.activation(ax[:], a_sb[:], AF.Exp)
        rs = sb.tile([128, H], F32, tag="rs")
        nc.vector.tensor_reduce(rs[:], ax[:].rearrange("p (h j) -> p h j", h=H),
                                axis=mybir.AxisListType.X, op=ALU.add)
        rcp = sb.tile([128, H], F32, tag="rcp")
        nc.vector.reciprocal_approx_fast(out=rcp[:], in_=rs[:])
        nc.vector.tensor_mul(gr_all[:, gp, :].rearrange("p (h j) -> p h j", h=H),
                             gsig[:, gp, :].rearrange("p (h j) -> p h j", h=H),
                             _b0(rcp[:], 64))
        aT_a = ps_at_a.tile([64, CS], BF16, tag="aTa")
        aT_b = ps_at_b.tile([128, CS], BF16, tag="aTb")
        for h in range(H):
            nc.tensor.transpose(aT_a[:, h * 64:(h + 1) * 64],
                                ax[0:64, h * 64:(h + 1) * 64],
                                ident[0:64, 0:64], tile_position=(0, 0))
            nc.tensor.transpose(aT_b[64:128, h * 64:(h + 1) * 64],
                                ax[64:128, h * 64:(h + 1) * 64],
                                ident[64:128, 64:128], tile_position=(64, 64))
        nc.scalar.copy(aTs_all[0:64, gp, :], aT_a[:])
        nc.scalar.copy(aTs_all[64:128, gp, :], aT_b[64:128, :])

    with tc.tile_pool(name="ps_sc", bufs=2, space="PSUM") as ps_sc, \
         tc.tile_pool(name="ps_at_a", bufs=2, space="PSUM") as ps_at_a, \
         tc.tile_pool(name="ps_at_b", bufs=2, space="PSUM") as ps_at_b:
        prev = emit_d1_qk(0, ps_sc)
        for gp in range(RT):
            nxt = emit_d1_qk(gp + 1, ps_sc) if gp + 1 < RT else None
            emit_d1_rest(gp, *prev, ps_at_a, ps_at_b)
            prev = nxt

    # ================= D2: AV + gate + Wout + residual =================
    def emit_d2_av(gp, ps_o_a, ps_o_b):
        o_a = ps_o_a.tile([64, CS], F32, tag="oa")
        o_b = ps_o_b.tile([128, CS], F32, tag="ob")
        for h in range(H):
            nc.tensor.matmul(o_a[:, h * 64:(h + 1) * 64],
                             aTs_all[0:64, gp, h * 64:(h + 1) * 64],
                             vtm[0:64, gp, h * 64:(h + 1) * 64],
                             start=True, stop=True, tile_position=(0, 0))
            nc.tensor.matmul(o_b[64:128, h * 64:(h + 1) * 64],
                             aTs_all[64:128, gp, h * 64:(h + 1) * 64],
                             vtm[64:128, gp, h * 64:(h + 1) * 64],
                             start=True, stop=True, tile_position=(64, 64))
        return o_a, o_b

    def emit_d2_rest(gp, o_a, o_b, ps_og, ps_w):
        og = sb.tile([128, CS], BF16, tag="og")
        nc.vector.tensor_mul(og[0:64, :].rearrange("p (h j) -> p h j", h=H),
                             o_a[:].rearrange("p (h j) -> p h j", h=H),
                             gr_all[0:64, gp, :].rearrange("p (h j) -> p h j", h=H))
        nc.vector.tensor_mul(og[64:128, :].rearrange("p (h j) -> p h j", h=H),
                             o_b[64:128, :].rearrange("p (h j) -> p h j", h=H),
                             gr_all[64:128, gp, :].rearrange("p (h j) -> p h j", h=H))
        ogT = ps_og.tile([128, CS], BF16, tag="ogT")
        for c in range(4):
            nc.tensor.transpose(ogT[:, c * 128:(c + 1) * 128],
                                og[:, c * 128:(c + 1) * 128], ident[:])
        ogs = sb.tile([128, 4, 128], BF16, tag="ogs")
        nc.scalar.copy(ogs[:].rearrange("p k c -> p (k c)"), ogT[:])
        ptw = ps_w.tile([128, CS], F32, tag="ptw")
        for k in range(4):
            nc.tensor.matmul(ptw[:], ogs[:, k, :], wout[:, k, :],
                             start=(k == 0), stop=(k == 3))
        nc.vector.tensor_add(h_sb[:, gp, :], ptw[:], re_sb[:, gp, :])

    with tc.tile_pool(name="ps_o_a", bufs=2, space="PSUM") as ps_o_a, \
         tc.tile_pool(name="ps_o_b", bufs=2, space="PSUM") as ps_o_b, \
         tc.tile_pool(name="ps_og", bufs=2, space="PSUM") as ps_og, \
         tc.tile_pool(name="ps_w", bufs=2, space="PSUM") as ps_w:
        prev = emit_d2_av(0, ps_o_a, ps_o_b)
        for gp in range(RT):
            nxt = emit_d2_av(gp + 1, ps_o_a, ps_o_b) if gp + 1 < RT else None
            emit_d2_rest(gp, *prev, ps_og, ps_w)
            prev = nxt

    bd_stack.close()   # free B..D SBUF (qf/kf/vtm/zt rings/Pall/...) before E

    # ================= E: gather-conditioned transition =================
    with tc.tile_pool(name="ep", bufs=2) as ep, \
         tc.tile_pool(name="epw", bufs=1) as epw, \
         tc.tile_pool(name="ps_tt", bufs=1, space="PSUM") as ps_tt, \
         tc.tile_pool(name="ps_A", bufs=3, space="PSUM") as ps_A, \
         tc.tile_pool(name="ps_B", bufs=3, space="PSUM") as ps_B, \
         tc.tile_pool(name="ps_wb", bufs=1, space="PSUM") as ps_wb:
        w1 = epw.tile([128, 4, 2 * CS], BF16)
        nc.sync.dma_start(w1[:], t["w1"][:])
        w2 = epw.tile([128, 4, 2 * CS], BF16)
        nc.sync.dma_start(w2[:], t["w2"][:])
        wb = epw.tile([128, 8, CS], BF16)
        nc.sync.dma_start(wb[:], t["wb"][:])
        tT = epw.tile([128, 4, NT], BF16)
        bb = epw.tile([128, 8, NT], BF16)

        mve, rstde = _batch_stats(nc, sb, lambda r: h_sb[:, r, :], RT, "ee")
        for r in range(RT):
            t0 = ep.tile([128, CS], BF16, tag="t0")
            nc.vector.tensor_scalar(out=t0[:], in0=h_sb[:, r, :],
                                    scalar1=mve[:, r, 0:1],
                                    scalar2=rstde[:, r:r + 1],
                                    op0=ALU.subtract, op1=ALU.mult)
            t1 = ep.tile([128, CS], BF16, tag="t1")
            nc.vector.tensor_mul(t1[:], t0[:], gth[:, r, 0:CS])
            t2 = ep.tile([128, CS], BF16, tag="t2")
            nc.vector.tensor_add(t2[:], t1[:], gth[:, r, CS:2 * CS])
            tp = ps_tt.tile([128, CS], BF16, tag="tt")
            for c in range(4):
                nc.tensor.transpose(tp[:, c * 128:(c + 1) * 128],
                                    t2[:, c * 128:(c + 1) * 128], ident[:])
            nc.vector.tensor_copy(
                tT[:, :, r * 128:(r + 1) * 128],
                tp[:].rearrange("p (k c) -> p k c", k=4))

        for n in range(2):
            for m in range(8):
                pA = ps_A.tile([128, CS], F32, tag="pA")
                for k in range(4):
                    nc.tensor.matmul(pA[:], w1[:, k, m * 128:(m + 1) * 128],
                                     tT[:, k, n * 512:(n + 1) * 512],
                                     start=(k == 0), stop=(k == 3))
                pB = ps_B.tile([128, CS], F32, tag="pB")
                for k in range(4):
                    nc.tensor.matmul(pB[:], w2[:, k, m * 128:(m + 1) * 128],
                                     tT[:, k, n * 512:(n + 1) * 512],
                                     start=(k == 0), stop=(k == 3))
                u1s = ep.tile([128, 512], BF16, tag="u1s")
                nc.scalar.activation(u1s[:], pA[:], AF.Sigmoid)
                u1 = ep.tile([128, 512], F32, tag="u1")
                nc.vector.tensor_mul(u1[:], u1s[:], pA[:])
                nc.vector.tensor_mul(bb[:, m, n * 512:(n + 1) * 512], u1[:], pB[:])
            for r in range(n * 4, n * 4 + 4):
                ptb = ps_wb.tile([128, CS], F32, tag="ptb")
                for k in range(8):
                    nc.tensor.matmul(ptb[:], bb[:, k, r * 128:(r + 1) * 128],
                                     wb[:, k, :], start=(k == 0), stop=(k == 7))
                tr = ep.tile([128, CS], F32, tag="tr")
                nc.vector.tensor_mul(tr[:], ptb[:], gth[:, r, 2 * CS:3 * CS])
                out_t = ep.tile([128, CS], F32, tag="out_t")
                nc.vector.tensor_add(out_t[:], tr[:], h_sb[:, r, :])
                nc.sync.dma_start(t["out"][r * 128:(r + 1) * 128, :], out_t[:])


def build(flags):
    key = ("v2", flags)
    if key in _CACHE:
        return _CACHE[key]
    nc = bacc.Bacc("TRN2", target_bir_lowering=False, debug=False)
    t = _declare(nc)
    with tile.TileContext(nc) as tc:
        with ExitStack() as ctx:
            _emit(ctx, tc, t, flags)
    nc.compile()
    _CACHE[key] = nc
    return nc


def prep_core_inputs(inputs, core):
    """Host-side slicing + weight folding for one core."""
    b = core // 4
    g0 = (core % 4) * NBLK
    r0 = g0 * BLK

    f = lambda k: np.asarray(inputs[k], np.float32)
    ln_w, ln_b = f("ln_w"), f("ln_b")
    sc = 1.0 / np.sqrt(CH)

    def fold(w, scale=1.0):
        return ln_w[:, None] * np.asarray(w, np.float32) * scale

    def foldb(w, scale=1.0):
        return (ln_b @ np.asarray(w, np.float32)) * scale

    Wkv = f("Wkv")
    wq_h, bq_h = fold(inputs["Wq"], sc), foldb(inputs["Wq"], sc)
    wk_h, bk_h = fold(Wkv[:, :CS]), foldb(Wkv[:, :CS])
    wv_h, bv_h = fold(Wkv[:, CS:]), foldb(Wkv[:, CS:])
    wg_h, bg_h = fold(inputs["Wgate"]), foldb(inputs["Wgate"])
    if np.any(bv_h) or np.any(bg_h):
        raise NotImplementedError("nonzero folded v/gate bias unsupported")

    cw = f("adaln_cond_w")
    wada_h = np.concatenate(
        [cw[:, None] * f("W_ada_gate"), cw[:, None] * f("W_ada_bias"),
         cw[:, None] * f("W_tgate")], axis=1)
    bada_h = np.concatenate(
        [f("b_ada_gate"), np.zeros(CS, np.float32), f("b_tgate")]).astype(np.float32)

    # wbs': fold the mean-correction into the weights (bias = P'*rstd);
    # col 8 of the z-pass = mean, col 32+9 of the z^2 pass = E[z^2]
    wbias = f("bias_ln_w")[:, None] * f("Wbias")      # [128, 8]
    wbs_h = np.zeros((CZ, 64), np.float32)
    wbs_h[:, :H] = wbias - wbias.sum(0, keepdims=True) / CZ
    wbs_h[:, 8] = 1.0 / CZ
    wbs_h[:, 32 + 9] = 1.0 / CZ

    def ktile(w, kt):
        w = np.asarray(w, np.float32)
        return np.ascontiguousarray(
            w.reshape(kt, 128, w.shape[1]).transpose(1, 0, 2)).astype(BF)

    # framepair: [16, 64, 64, 128] -> [16, 128, 4096] bf16
    fp = np.asarray(inputs["framepair_embed"][b, g0:g0 + NBLK], np.float32)
    zT = np.ascontiguousarray(
        fp.reshape(NBLK, BLK * BLK, CZ).transpose(0, 2, 1)).astype(BF)

    idx = np.asarray(inputs["rigids_to_res_idx"][b, r0:r0 + NT]).astype(np.int16)
    idx_w = np.empty((128, NT // 16), np.int16)
    for p in range(16):
        idx_w[p] = idx[p::16]
    idx_w[16:] = np.tile(idx_w[:16], (7, 1))

    btg = f("b_tgate")
    btg_const = float(btg[0]) if np.all(btg == btg[0]) else None
    has_btg = bool(np.any(btg))

    return {
        "re": np.ascontiguousarray(inputs["rigids_embed"][b, r0:r0 + NT]).astype(BF),
        "zT": zT,
        "s": np.ascontiguousarray(inputs["s"][b]).astype(np.float32),
        "idx": idx_w,
        "wq": ktile(wq_h, 4), "wk": ktile(wk_h, 4), "wv": ktile(wv_h, 4),
        "wg": ktile(wg_h, 4), "wout": ktile(inputs["Wout"], 4),
        "w1": ktile(inputs["W1"], 4), "w2": ktile(inputs["W2"], 4),
        "wb": ktile(inputs["Wb"], 8), "wada": ktile(wada_h, 3),
        "wbs": wbs_h.astype(BF),
        "bq": np.ascontiguousarray(bq_h.reshape(4, 128).T),
        "bk": np.ascontiguousarray(bk_h.reshape(4, 128).T),
        "bada": bada_h,
    }, (bool(np.any(bq_h)), bool(np.any(bk_h)), bool(np.any(f("b_ada_gate"))),
        btg_const, has_btg)


def kernel(**inputs):
    mask = np.asarray(inputs["rigids_mask"])
    if not np.all(mask == 1.0):
        print("WARNING: rigids_mask not all ones; kernel assumes ones", file=sys.stderr)

    in_maps, flags = [], None
    for core in range(NCORES):
        m, flags = prep_core_inputs(inputs, core)
        in_maps.append(m)

    nc = build(flags)
    res = run_bass_kernel_spmd(nc, in_maps, core_ids=list(range(NCORES)))

    out = np.empty((B, N, CS), np.float32)
    for core in range(NCORES):
        b = core // 4
        r0 = (core % 4) * NT
        out[b, r0:r0 + NT] = res.results[core]["out"]
    return out
